# revision 18
# baseline (speedup 1.0000x reference)
"""Bahdanau additive attention on 8 TRN2 NeuronCores (Bass/Tile via axon PJRT).

Reference (per batch b):
  Q = hs[b] @ W.T ; K = hs[b] @ U.T                      (S,H)
  scores[q,k] = sum_h v[h] * tanh(Q[q,h] + K[k,h])       (S,S)
  out[b] = softmax(scores, axis=-1) @ hs[b]              (S,H)

Sharding: core c owns batch c//2, query rows [(c%2)*256, +256). Each core
uploads only its 256 query rows of hidden plus a 1/8 shard of W.T/U.T;
on-device AllGathers reconstruct the full hidden[b] (within core pairs)
and the full weights (across all 8) to keep host->device bytes minimal.

Per-core pipeline (H=256 as two 128-partition halves):
  - PE transposes hidden -> hidT/hqT; projections on PE (f32r) give
    KpT/QpT with h_out on partitions.
  - Scores in half-chunks of 8 queries: one DVE broadcast add
    [128,2,8,512], one ACT Tanh -> bf16, then per (q, half) a PE matvec
    (lhsT = v half, f32r) reducing over the 128 h partitions, landing
    each query's 512 scores in its own PSUM (partition-group, bank)
    slot; a single strided DMA compacts 8 rows into the (128q, 512k)
    scores tile.
  - Softmax: exp with fused accum_out row sums (scores bounded by
    sum|v|, so no max shift), reciprocal; normalization folded into the
    output scaling.
  - Context: PE-transpose of the weights, then 4 accumulating f32r
    matmuls against hidden, rows scaled by 1/sum.
"""

import numpy as np

B, S, H = 4, 512, 256
NCORES = 8
QPC = (B * S) // NCORES  # 256 queries per core
HP = 128
CQ = 8                   # queries per half-chunk (DVE/ACT/PE unit)
NHC = QPC // CQ          # 32 half-chunks
KC = S // 128            # 4 key chunks
WSH = H // NCORES        # 32 rows of W.T per core

_CACHE = {}


def _build(reps=1, skip=()):
    import concourse.bass as bass
    import concourse.tile as tile
    import concourse.mybir as mybir
    from concourse import bacc
    from concourse.masks import make_identity
    from contextlib import ExitStack

    f32 = mybir.dt.float32
    f32r = mybir.dt.float32r
    bf16 = mybir.dt.bfloat16
    AF = mybir.ActivationFunctionType
    ADD = mybir.AluOpType.add

    nc = bacc.Bacc("TRN2", target_bir_lowering=False, debug=False)

    hidhalf = nc.declare_dram_parameter("hidhalf", [QPC, H], f32, isOutput=False)
    WTsh = nc.declare_dram_parameter("WTsh", [WSH, H], f32, isOutput=False)
    UTsh = nc.declare_dram_parameter("UTsh", [WSH, H], f32, isOutput=False)
    vpack = nc.declare_dram_parameter("vpack", [HP, 2], f32, isOutput=False)
    out = nc.declare_dram_parameter("out", [QPC, H], f32, isOutput=True)
    dbg = None
    if "dbg" in skip:
        dbg = nc.declare_dram_parameter("dbg", [QPC, S], f32, isOutput=True)

    hh_b = nc.dram_tensor("hh_b", [QPC, H], f32)
    wt_b = nc.dram_tensor("wt_b", [WSH, H], f32)
    ut_b = nc.dram_tensor("ut_b", [WSH, H], f32)
    hid_full = nc.dram_tensor("hid_full", [S, H], f32)
    WT_full = nc.dram_tensor("WT_full", [H, H], f32, addr_space="Shared")
    UT_full = nc.dram_tensor("UT_full", [H, H], f32, addr_space="Shared")

    with tile.TileContext(nc) as tc, ExitStack() as ctx:
        sg = ctx.enter_context(tc.tile_pool(name="sg", bufs=1))
        addp = ctx.enter_context(tc.tile_pool(name="addp", bufs=2))
        tanp = ctx.enter_context(tc.tile_pool(name="tanp", bufs=2))
        scp = ctx.enter_context(tc.tile_pool(name="scp", bufs=2))
        scbp = ctx.enter_context(tc.tile_pool(name="scbp", bufs=2))
        auxp = ctx.enter_context(tc.tile_pool(name="auxp", bufs=2))
        outp = ctx.enter_context(tc.tile_pool(name="outp", bufs=2))
        psm = ctx.enter_context(tc.tile_pool(name="psm", bufs=2, space="PSUM"))
        psc = ctx.enter_context(tc.tile_pool(name="psc", bufs=1, space="PSUM"))

        # own query rows + v (plain loads)
        hq = []
        for t2 in range(2):
            t = sg.tile([HP, H], f32, tag=f"hq{t2}")
            nc.sync.dma_start(out=t, in_=hidhalf[t2 * HP : (t2 + 1) * HP, :])
            hq.append(t)
        sv = sg.tile([HP, 2], f32, tag="v")
        nc.sync.dma_start(out=sv, in_=vpack[:])
        svb = sg.tile([HP, 2], bf16, tag="vb")
        nc.vector.tensor_copy(svb, sv)

        # gather full hidden (pairs) + full weights (all cores)
        sb_hid = [sg.tile([HP, H], f32, tag=f"hid{k}", name=f"hid{k}") for k in range(KC)]
        sb_WT = [sg.tile([HP, H], f32, tag=f"WT{i}", name=f"WT{i}") for i in range(2)]
        sb_UT = [sg.tile([HP, H], f32, tag=f"UT{i}", name=f"UT{i}") for i in range(2)]
        with tc.tile_critical():
            with (
                nc.semaphore("dmasem") as dmasem,
                nc.semaphore("ccsem") as ccsem,
            ):
                nc.gpsimd.dma_start(out=hh_b[:], in_=hidhalf[:]).then_inc(dmasem, 16)
                nc.gpsimd.dma_start(out=wt_b[:], in_=WTsh[:]).then_inc(dmasem, 16)
                nc.gpsimd.dma_start(out=ut_b[:], in_=UTsh[:]).then_inc(dmasem, 16)
                nc.gpsimd.wait_ge(dmasem, 48)
                nc.gpsimd.collective_compute(
                    "AllGather", mybir.AluOpType.bypass,
                    replica_groups=[[0, 1], [2, 3], [4, 5], [6, 7]],
                    ins=[hh_b[:]], outs=[hid_full[:]],
                ).then_inc(ccsem, 1)
                nc.gpsimd.collective_compute(
                    "AllGather", mybir.AluOpType.bypass,
                    replica_groups=[list(range(NCORES))],
                    ins=[wt_b[:]], outs=[WT_full[:]],
                ).then_inc(ccsem, 1)
                nc.gpsimd.collective_compute(
                    "AllGather", mybir.AluOpType.bypass,
                    replica_groups=[list(range(NCORES))],
                    ins=[ut_b[:]], outs=[UT_full[:]],
                ).then_inc(ccsem, 1)
                nc.gpsimd.wait_ge(ccsem, 3)
                for k in range(KC):
                    nc.gpsimd.dma_start(
                        out=sb_hid[k], in_=hid_full[k * HP : (k + 1) * HP, :]
                    ).then_inc(dmasem, 16)
                for i in range(2):
                    nc.gpsimd.dma_start(
                        out=sb_WT[i], in_=WT_full[i * HP : (i + 1) * HP, :]
                    ).then_inc(dmasem, 16)
                    nc.gpsimd.dma_start(
                        out=sb_UT[i], in_=UT_full[i * HP : (i + 1) * HP, :]
                    ).then_inc(dmasem, 16)
                nc.gpsimd.wait_ge(dmasem, 176)

        ident = sg.tile([HP, HP], f32, tag="ident")
        make_identity(nc, ident)

        for rep in range(reps):
            # hidT (h on partitions, all 512 tokens) and hqT (own 256 queries)
            sb_hidT = []
            sb_hqT = []
            for hc in range(2):
                ps = psm.tile([HP, S], f32, tag="ps")
                for k in range(KC):
                    nc.tensor.transpose(
                        ps[:, k * HP : (k + 1) * HP],
                        sb_hid[k][:, hc * HP : (hc + 1) * HP], ident)
                t = auxp.tile([HP, S], f32, tag=f"hidT{hc}")
                nc.vector.tensor_copy(t, ps)
                sb_hidT.append(t)
            for hc in range(2):
                ps = psm.tile([HP, H], f32, tag="ps")
                for t2 in range(2):
                    nc.tensor.transpose(
                        ps[:, t2 * HP : (t2 + 1) * HP],
                        hq[t2][:, hc * HP : (hc + 1) * HP], ident)
                t = auxp.tile([HP, H], f32, tag=f"hqT{hc}")
                nc.vector.tensor_copy(t, ps)
                sb_hqT.append(t)

            # projections: KpT[o,k] / QpT[o,q] with h_out on partitions,
            # both halves packed into one tile along a free axis (f32r PE)
            Kp2 = auxp.tile([HP, 2, S], f32, tag="Kp2")
            Qp2 = auxp.tile([HP, 2, H], f32, tag="Qp2")
            for oc in range(2):
                ps = psm.tile([HP, S], f32, tag="ps")
                for hc in range(2):
                    nc.tensor.matmul(
                        ps, lhsT=sb_UT[hc][:, oc * HP : (oc + 1) * HP],
                        rhs=sb_hidT[hc], start=(hc == 0), stop=(hc == 1))
                nc.vector.tensor_copy(Kp2[:, oc], ps)
            for oc in range(2):
                ps = psm.tile([HP, H], f32, tag="ps")
                for hc in range(2):
                    nc.tensor.matmul(
                        ps, lhsT=sb_WT[hc][:, oc * HP : (oc + 1) * HP],
                        rhs=sb_hqT[hc], start=(hc == 0), stop=(hc == 1))
                nc.vector.tensor_copy(Qp2[:, oc], ps)

            for qt in range(QPC // 128):
                sc = scp.tile([128, S], f32, tag="sc")
                for hc2 in range(128 // CQ):
                    q0 = qt * 128 + hc2 * CQ
                    # A[p, half, q, k] = Kp2[p, half, k] + Qp2[p, half, q0+q]
                    A = addp.tile([HP, 2, CQ, S], f32, tag="A")
                    k_b = bass.AP(
                        tensor=Kp2.tensor, offset=Kp2.offset,
                        ap=[Kp2.ap[0], [S, 2], [0, CQ], [1, S]])
                    q_b = bass.AP(
                        tensor=Qp2.tensor, offset=Qp2.offset + q0,
                        ap=[Qp2.ap[0], [H, 2], [1, CQ], [0, S]])
                    nc.vector.tensor_tensor(out=A, in0=k_b, in1=q_b, op=ADD)
                    T = tanp.tile([HP, 2, CQ, S], bf16, tag="T")
                    nc.scalar.activation(T, A, AF.Tanh)
                    # per-(q, half) PE matvec reduce over h partitions:
                    # out[partition 32*(q//4), bank q%4] accumulates both halves
                    pscore = psc.tile([HP, 4, S], f32, tag="pscore")
                    for q in range(CQ):
                        po = 32 * (q // 4)
                        for half in range(2):
                            nc.tensor.matmul(
                                pscore[po : po + 1, q % 4, :],
                                lhsT=svb[:, half : half + 1],
                                rhs=T[:, half, q, :],
                                start=(half == 0), stop=(half == 1))
                    # evacuate PSUM -> SBUF on ACT (rows {0,32} live), then
                    # compact 8 rows (partition group, bank) -> sc rows via DMA
                    scb = scbp.tile([HP, 4, S], f32, tag="scb")
                    nc.scalar.copy(scb, pscore)
                    for g in range(2):
                        nc.sync.dma_start(
                            out=sc[hc2 * CQ + 4 * g : hc2 * CQ + 4 * (g + 1), :],
                            in_=scb[32 * g : 32 * g + 1, :, :])


                if dbg is not None and rep == 0:
                    nc.sync.dma_start(
                        out=dbg[qt * 128 : (qt + 1) * 128, :], in_=sc)
                # softmax pieces (scores bounded by sum|v| ~ 13: exp-safe)
                wts = scp.tile([128, S], f32, tag="wts")
                ssum = auxp.tile([128, 1], f32, tag="ssum")
                nc.scalar.activation(wts, sc, AF.Exp, accum_out=ssum)
                rinv = auxp.tile([128, 1], f32, tag="rinv")
                nc.vector.reciprocal(rinv, ssum)

                # context: wtsT chunks via PE transpose, then 4 matmuls
                ps_t = psm.tile([HP, S], f32, tag="ps")
                for kc in range(KC):
                    nc.tensor.transpose(
                        ps_t[:, kc * HP : (kc + 1) * HP],
                        wts[:, kc * HP : (kc + 1) * HP], ident)
                wtsT = scp.tile([128, S], f32, tag="wtsT")
                nc.vector.tensor_copy(wtsT, ps_t)
                pctx = psm.tile([128, H], f32, tag="ps")
                for kc in range(KC):
                    nc.tensor.matmul(
                        pctx, lhsT=wtsT[:, kc * HP : (kc + 1) * HP],
                        rhs=sb_hid[kc],
                        start=(kc == 0), stop=(kc == KC - 1))
                octx = outp.tile([128, H], f32, tag="octx")
                nc.vector.tensor_scalar_mul(octx, pctx, rinv)
                nc.sync.dma_start(out=out[qt * 128 : (qt + 1) * 128, :], in_=octx)

    nc.compile()
    return nc


def _get(reps=1, skip=()):
    key = (reps, tuple(skip))
    if key not in _CACHE:
        _CACHE[key] = _build(reps, skip)
    return _CACHE[key]


def _in_maps(hs, W, U, v):
    hs = np.asarray(hs, np.float32)
    WTh = np.ascontiguousarray(np.asarray(W, np.float32).T)
    UTh = np.ascontiguousarray(np.asarray(U, np.float32).T)
    vp = np.ascontiguousarray(np.asarray(v, np.float32).reshape(2, HP).T)
    maps = []
    for c in range(NCORES):
        b, qh = divmod(c, 2)
        maps.append({
            "hidhalf": np.ascontiguousarray(hs[b, qh * QPC : (qh + 1) * QPC]),
            "WTsh": np.ascontiguousarray(WTh[c * WSH : (c + 1) * WSH]),
            "UTsh": np.ascontiguousarray(UTh[c * WSH : (c + 1) * WSH]),
            "vpack": vp,
        })
    return maps


def run(hidden_states, W, U, v, reps=1, skip=()):
    from concourse.bass_utils import run_bass_kernel_spmd

    nc = _get(reps, skip)
    res = run_bass_kernel_spmd(
        nc, _in_maps(hidden_states, W, U, v), core_ids=list(range(NCORES)))
    ctxout = np.empty((B, S, H), np.float32)
    for c in range(NCORES):
        b, qh = divmod(c, 2)
        ctxout[b, qh * QPC : (qh + 1) * QPC] = res.results[c]["out"]
    return ctxout


def kernel(**inputs):
    return run(inputs["hidden_states"], inputs["W"], inputs["U"], inputs["v"])


# revision 22
# speedup vs baseline: 5.9715x; 5.9715x over previous
"""Bahdanau additive attention on 8 TRN2 NeuronCores (Bass/Tile via axon PJRT).

Reference (per batch b):
  Q = hs[b] @ W.T ; K = hs[b] @ U.T                      (S,H)
  scores[q,k] = sum_h v[h] * tanh(Q[q,h] + K[k,h])       (S,S)
  out[b] = softmax(scores, axis=-1) @ hs[b]              (S,H)

Sharding: core c owns batch c//2, query rows [(c%2)*256, +256). The host
uploads per core: hidT (hs[b].T, h-half-major), hqT (own query columns),
hidb (hs[b] in bf16, k-block-major), WTp/UTp (W.T/U.T h-half-major), v.

This environment is dominated by a ~50-80us fixed cost PER INSTRUCTION
(per engine), so the kernel minimizes instruction count and spreads
stages across engines:
  - Projections: 8 PE matmuls into PSUM-resident Kp/Qp (no SBUF copies;
    the DVE reads PSUM directly).
  - Scores in chunks of CQ=32 queries: one DVE broadcast-add
    [128,2,32,512] -> bf16 (h on partitions, h-half/q/k on free), one
    in-place ACT tanh, v-multiply (split DVE/ACT), one in-place DVE
    half-combine, one gpsimd partition_all_reduce (f32 accum, fp16 out),
    one compaction DMA into the (128q, 512k) scores tile.
  - Softmax: one exp with fused accum_out row sums (scores bounded by
    sum|v|: no max shift), one reciprocal; normalization folded into the
    output scaling.
  - Context: 4 DMA-transposes (bf16) replace PE transposes, then 4
    accumulating bf16 PE matmuls against hidb, rows scaled by 1/sum.
"""

import numpy as np

B, S, H = 4, 512, 256
NCORES = 8
QPC = (B * S) // NCORES  # 256 queries per core
HP = 128
CQ = 32                  # queries per chunk
NCH = QPC // CQ          # 8 chunks
KC = S // 128            # 4 key blocks
NDVE_VMUL = 2            # chunks whose v-multiply runs on DVE (rest on ACT)

_CACHE = {}


def _build(reps=1, skip=()):
    import concourse.bass as bass
    import concourse.tile as tile
    import concourse.mybir as mybir
    from concourse import bacc, bass_isa
    from contextlib import ExitStack

    f32 = mybir.dt.float32
    bf16 = mybir.dt.bfloat16
    fp16 = mybir.dt.float16
    AF = mybir.ActivationFunctionType
    ADD = mybir.AluOpType.add
    MUL = mybir.AluOpType.mult

    nc = bacc.Bacc("TRN2", target_bir_lowering=False, debug=False)

    hidT_u = nc.declare_dram_parameter("hidT_u", [HP, 2, S], f32, isOutput=False)
    hqT_u = nc.declare_dram_parameter("hqT_u", [HP, 2, QPC], f32, isOutput=False)
    hidb_u = nc.declare_dram_parameter("hidb_u", [HP, KC, H], bf16, isOutput=False)
    WTp_u = nc.declare_dram_parameter("WTp_u", [HP, 2, H], f32, isOutput=False)
    UTp_u = nc.declare_dram_parameter("UTp_u", [HP, 2, H], f32, isOutput=False)
    vpack = nc.declare_dram_parameter("vpack", [HP, 2], f32, isOutput=False)
    out = nc.declare_dram_parameter("out", [QPC, H], f32, isOutput=True)
    dbg = None
    if "dbg" in skip:
        dbg = nc.declare_dram_parameter("dbg", [QPC, S], f32, isOutput=True)

    with tile.TileContext(nc) as tc, ExitStack() as ctx:
        sg = ctx.enter_context(tc.tile_pool(name="sg", bufs=1))
        addp = ctx.enter_context(tc.tile_pool(name="addp", bufs=2))
        redp = ctx.enter_context(tc.tile_pool(name="redp", bufs=1))
        scp = ctx.enter_context(tc.tile_pool(name="scp", bufs=2))
        wtp = ctx.enter_context(tc.tile_pool(name="wtp", bufs=2))
        auxp = ctx.enter_context(tc.tile_pool(name="auxp", bufs=2))
        outp = ctx.enter_context(tc.tile_pool(name="outp", bufs=2))
        psA = ctx.enter_context(tc.tile_pool(name="psA", bufs=1, space="PSUM"))
        psC = ctx.enter_context(tc.tile_pool(name="psC", bufs=2, space="PSUM"))

        hidT = sg.tile([HP, 2, S], f32, tag="hidT")
        nc.sync.dma_start(out=hidT, in_=hidT_u[:])
        hqT = sg.tile([HP, 2, QPC], f32, tag="hqT")
        nc.sync.dma_start(out=hqT, in_=hqT_u[:])
        hidb = sg.tile([HP, KC, H], bf16, tag="hidb")
        nc.sync.dma_start(out=hidb, in_=hidb_u[:])
        WTp = sg.tile([HP, 2, H], f32, tag="WTp")
        nc.sync.dma_start(out=WTp, in_=WTp_u[:])
        UTp = sg.tile([HP, 2, H], f32, tag="UTp")
        nc.sync.dma_start(out=UTp, in_=UTp_u[:])
        sv = sg.tile([HP, 2], f32, tag="v")
        nc.sync.dma_start(out=sv, in_=vpack[:])

        for rep in range(reps):
            # projections into PSUM: Kp[p, oc, k], Qp[p, oc, q] (h_out on
            # partitions as oc*128+p; h_in contracted as hc*128+p)
            KpP = psA.tile([HP, 2, S], f32, tag="KpP")
            QpP = psA.tile([HP, 2, S], f32, tag="QpP")
            for oc in range(2):
                for hc in range(2):
                    nc.tensor.matmul(
                        KpP[:, oc, :],
                        lhsT=UTp[:, hc, oc * HP : (oc + 1) * HP],
                        rhs=hidT[:, hc, :], start=(hc == 0), stop=(hc == 1))
            for oc in range(2):
                for hc in range(2):
                    nc.tensor.matmul(
                        QpP[:, oc, 0:QPC],
                        lhsT=WTp[:, hc, oc * HP : (oc + 1) * HP],
                        rhs=hqT[:, hc, :], start=(hc == 0), stop=(hc == 1))
            Qs = auxp.tile([HP, 2, QPC], f32, tag="Qs")
            nc.scalar.copy(Qs, QpP[:, :, 0:QPC])

            for qt in range(QPC // 128):
                sc = scp.tile([128, S], fp16, tag="sc")
                for c4 in range(128 // CQ):
                    ch = qt * (128 // CQ) + c4
                    q0 = ch * CQ
                    # A[p, half, q, k] = Kp[p, half, k] + Qp[p, half, q0+q]
                    A = addp.tile([HP, 2, CQ, S], bf16, tag="A")
                    k_b = bass.AP(
                        tensor=KpP.tensor, offset=KpP.offset,
                        ap=[KpP.ap[0], [S, 2], [0, CQ], [1, S]])
                    q_b = bass.AP(
                        tensor=Qs.tensor, offset=Qs.offset + q0,
                        ap=[Qs.ap[0], [QPC, 2], [1, CQ], [0, S]])
                    nc.vector.tensor_tensor(out=A, in0=k_b, in1=q_b, op=ADD)
                    nc.scalar.activation(A, A, AF.Tanh)
                    if ch < NDVE_VMUL:
                        v_b = bass.AP(
                            tensor=sv.tensor, offset=sv.offset,
                            ap=[sv.ap[0], [1, 2], [0, CQ], [0, S]])
                        nc.vector.tensor_tensor(out=A, in0=A, in1=v_b, op=MUL)
                    else:
                        for half in range(2):
                            nc.scalar.mul(A[:, half], A[:, half],
                                          sv[:, half : half + 1])
                    nc.vector.tensor_tensor(
                        out=A[:, 0], in0=A[:, 0], in1=A[:, 1], op=ADD)
                    R = redp.tile([HP, CQ, S], fp16, tag="R")
                    nc.gpsimd.partition_all_reduce(
                        R.rearrange("p q k -> p (q k)"),
                        A[:, 0].rearrange("p q k -> p (q k)"),
                        channels=HP, reduce_op=bass_isa.ReduceOp.add)
                    nc.sync.dma_start(
                        out=sc[c4 * CQ : (c4 + 1) * CQ, :], in_=R[0:1, :, :])

                if dbg is not None and rep == 0:
                    dsc = auxp.tile([128, S], f32, tag="dsc")
                    nc.vector.tensor_copy(dsc, sc)
                    nc.sync.dma_start(
                        out=dbg[qt * 128 : (qt + 1) * 128, :], in_=dsc)

                # softmax (scores bounded by sum|v| ~ 13: exp-safe, no shift)
                wts = wtp.tile([128, S], bf16, tag="wts")
                ssum = auxp.tile([128, 1], f32, tag="ssum")
                nc.scalar.activation(wts, sc, AF.Exp, accum_out=ssum)
                rinv = auxp.tile([128, 1], f32, tag="rinv")
                nc.vector.reciprocal(rinv, ssum)

                # context: DMA-transpose wts (bf16), 4 accumulating matmuls
                wtsT = wtp.tile([128, KC, 128], bf16, tag="wtsT")
                for kb in range(KC):
                    nc.sync.dma_start(
                        out=wtsT[:, kb, :],
                        in_=wts[:, kb * HP : (kb + 1) * HP], transpose=True)
                pctx = psC.tile([128, H], f32, tag="pctx")
                for kb in range(KC):
                    nc.tensor.matmul(
                        pctx, lhsT=wtsT[:, kb, :], rhs=hidb[:, kb, :],
                        start=(kb == 0), stop=(kb == KC - 1))
                octx = outp.tile([128, H], f32, tag="octx")
                nc.vector.tensor_scalar_mul(octx, pctx, rinv)
                nc.sync.dma_start(out=out[qt * 128 : (qt + 1) * 128, :], in_=octx)

    nc.compile()
    return nc


def _get(reps=1, skip=()):
    key = (reps, tuple(skip))
    if key not in _CACHE:
        _CACHE[key] = _build(reps, skip)
    return _CACHE[key]


def _in_maps(hs, W, U, v):
    hs = np.asarray(hs, np.float32)
    WTh = np.asarray(W, np.float32).T  # [h_in, h_out]
    UTh = np.asarray(U, np.float32).T
    WTp = np.ascontiguousarray(WTh.reshape(2, HP, H).transpose(1, 0, 2))
    UTp = np.ascontiguousarray(UTh.reshape(2, HP, H).transpose(1, 0, 2))
    vp = np.ascontiguousarray(np.asarray(v, np.float32).reshape(2, HP).T)
    maps = []
    for c in range(NCORES):
        b, qh = divmod(c, 2)
        hT = hs[b].T  # [H, S]
        hidT = np.ascontiguousarray(hT.reshape(2, HP, S).transpose(1, 0, 2))
        hqT = np.ascontiguousarray(
            hT[:, qh * QPC : (qh + 1) * QPC].reshape(2, HP, QPC).transpose(1, 0, 2))
        hidb = np.ascontiguousarray(hs[b].reshape(KC, HP, H).transpose(1, 0, 2))
        maps.append({
            "hidT_u": hidT,
            "hqT_u": hqT,
            "hidb_u": _to_bf16(hidb),
            "WTp_u": WTp,
            "UTp_u": UTp,
            "vpack": vp,
        })
    return maps


def _to_bf16(a):
    import ml_dtypes
    return a.astype(ml_dtypes.bfloat16)


def run(hidden_states, W, U, v, reps=1, skip=()):
    from concourse.bass_utils import run_bass_kernel_spmd

    nc = _get(reps, skip)
    res = run_bass_kernel_spmd(
        nc, _in_maps(hidden_states, W, U, v), core_ids=list(range(NCORES)))
    ctxout = np.empty((B, S, H), np.float32)
    for c in range(NCORES):
        b, qh = divmod(c, 2)
        ctxout[b, qh * QPC : (qh + 1) * QPC] = res.results[c]["out"]
    return ctxout


def kernel(**inputs):
    return run(inputs["hidden_states"], inputs["W"], inputs["U"], inputs["v"])


# revision 24
# speedup vs baseline: 13.1543x; 2.2029x over previous
"""Bahdanau attention, k-partitioned layout (v5). See kernel.py docstring.

Per core: batch b = c//2, queries [(c%2)*256, +256). Layout puts k (keys)
on partitions and h on the innermost free axis, so:
  - the h-reduction is a DVE free-axis tensor_reduce (no gpsimd big pass),
  - scores emerge as scT [k-part, q] = ready-made context-matmul lhsT
    (no transposes, no PSUM compaction),
  - softmax denominators are a tiny gpsimd all-reduce over k partitions.
Q is replicated across partitions once per q-tile (fold-DMA + gpsimd
partition_broadcast). All big tiles are persistent (no pool churn) and the
elementwise chain runs in place: this environment charges ~40us per
small/pool-cycled instruction but near-zero overhead for big in-place ones.
"""

import numpy as np

B, S, H = 4, 512, 256
NCORES = 8
QPC = (B * S) // NCORES  # 256
HP = 128
KC = S // 128            # 4 k-blocks
NQT = QPC // 128         # 2 q-tiles

_CACHE = {}


def _build(reps=1, skip=()):
    import concourse.bass as bass
    import concourse.tile as tile
    import concourse.mybir as mybir
    from concourse import bacc, bass_isa
    from contextlib import ExitStack

    f32 = mybir.dt.float32
    bf16 = mybir.dt.bfloat16
    AF = mybir.ActivationFunctionType
    ADD = mybir.AluOpType.add
    MUL = mybir.AluOpType.mult

    nc = bacc.Bacc("TRN2", target_bir_lowering=False, debug=False)

    hidT_u = nc.declare_dram_parameter("hidT_u", [HP, 2, S], f32, isOutput=False)
    hqT_u = nc.declare_dram_parameter("hqT_u", [HP, 2, QPC], f32, isOutput=False)
    hidb_u = nc.declare_dram_parameter("hidb_u", [HP, KC, H], f32, isOutput=False)
    WTp_u = nc.declare_dram_parameter("WTp_u", [HP, 2, H], f32, isOutput=False)
    UTp_u = nc.declare_dram_parameter("UTp_u", [HP, 2, H], f32, isOutput=False)
    vrep_u = nc.declare_dram_parameter("vrep_u", [HP, H], bf16, isOutput=False)
    out = nc.declare_dram_parameter("out", [QPC, H], f32, isOutput=True)
    dbg = None
    if "dbg" in skip:
        dbg = nc.declare_dram_parameter("dbg", [QPC, S], f32, isOutput=True)

    with tile.TileContext(nc) as tc, ExitStack() as ctx:
        sg = ctx.enter_context(tc.tile_pool(name="sg", bufs=1))
        psA = ctx.enter_context(tc.tile_pool(name="psA", bufs=1, space="PSUM"))

        hidT = sg.tile([HP, 2, S], f32, tag="hidT")
        nc.sync.dma_start(out=hidT, in_=hidT_u[:])
        hqT = sg.tile([HP, 2, QPC], f32, tag="hqT")
        nc.sync.dma_start(out=hqT, in_=hqT_u[:])
        hidb = sg.tile([HP, KC, H], f32, tag="hidb")
        nc.sync.dma_start(out=hidb, in_=hidb_u[:])
        WTp = sg.tile([HP, 2, H], f32, tag="WTp")
        nc.sync.dma_start(out=WTp, in_=WTp_u[:])
        UTp = sg.tile([HP, 2, H], f32, tag="UTp")
        nc.sync.dma_start(out=UTp, in_=UTp_u[:])
        vrep = sg.tile([HP, H], bf16, tag="vrep")
        nc.sync.dma_start(out=vrep, in_=vrep_u[:])

        # persistent work tiles
        X = sg.tile([HP, 128, H], bf16, tag="X")           # 64KB/part
        Qrep = sg.tile([HP, 128, H], bf16, tag="Qrep")     # 64KB/part
        Qs = sg.tile([HP, NQT, H], bf16, tag="Qs")
        scT = sg.tile([HP, NQT, KC, 128], f32, tag="scT")
        wtsT = sg.tile([HP, NQT, KC, 128], f32, tag="wtsT")
        ksum = sg.tile([HP, NQT, KC * 128], f32, tag="ksum")
        qsum = sg.tile([HP, NQT, 128], f32, tag="qsum")
        qsumT = sg.tile([HP, NQT], f32, tag="qsumT")
        rinv = sg.tile([HP, NQT], f32, tag="rinv")
        octx = sg.tile([HP, NQT, H], f32, tag="octx")

        KpP = psA.tile([HP, KC, S], f32, tag="KpP")    # 4 banks, 256 used/blk
        QnP = psA.tile([HP, NQT, S], f32, tag="QnP")   # 2 banks, 256 used/blk
        pctx = psA.tile([HP, S], f32, tag="pctx")      # 1 bank, 256 used

        for rep in range(reps):
            # K projection: Krow[k, h_out] per k-block (k on partitions)
            for kb in range(KC):
                for hc in range(2):
                    nc.tensor.matmul(
                        KpP[:, kb, 0:H],
                        lhsT=hidT[:, hc, kb * 128 : (kb + 1) * 128],
                        rhs=UTp[:, hc, :], start=(hc == 0), stop=(hc == 1))
            # Q projection: Qnat[q, h_out] per q-tile
            for qt in range(NQT):
                for hc in range(2):
                    nc.tensor.matmul(
                        QnP[:, qt, 0:H],
                        lhsT=hqT[:, hc, qt * 128 : (qt + 1) * 128],
                        rhs=WTp[:, hc, :], start=(hc == 0), stop=(hc == 1))
            nc.scalar.copy(Qs, QnP[:, :, 0:H])

            for qt in range(NQT):
                # replicate Q[q,h] of this q-tile to all partitions
                nc.sync.dma_start(
                    out=Qrep[0:1, :, :],
                    in_=Qs[:, qt, :])
                nc.gpsimd.partition_broadcast(
                    Qrep.rearrange("p q h -> p (q h)"),
                    Qrep[0:1].rearrange("p q h -> p (q h)"))
                for kb in range(KC):
                    k_b = bass.AP(
                        tensor=KpP.tensor, offset=KpP.offset + kb * S,
                        ap=[KpP.ap[0], [0, 128], [1, H]])
                    nc.vector.tensor_tensor(out=X, in0=k_b, in1=Qrep, op=ADD)
                    nc.scalar.activation(X, X, AF.Tanh)
                    v_b = bass.AP(
                        tensor=vrep.tensor, offset=vrep.offset,
                        ap=[vrep.ap[0], [0, 128], [1, H]])
                    nc.vector.tensor_tensor(out=X, in0=X, in1=v_b, op=MUL)
                    sl = scT[:, qt, kb, :]
                    nc.vector.tensor_reduce(
                        bass.AP(tensor=sl.tensor, offset=sl.offset,
                                ap=[*sl.ap, [1, 1]]), X,
                        axis=mybir.AxisListType.X, op=ADD)
                # softmax pieces: exp, k-partition sums, fold k-blocks
                nc.scalar.activation(wtsT[:, qt], scT[:, qt], AF.Exp)
                nc.gpsimd.partition_all_reduce(
                    ksum[:, qt], wtsT[:, qt].rearrange("p a b -> p (a b)"),
                    channels=HP, reduce_op=bass_isa.ReduceOp.add)
                kv = bass.AP(
                    tensor=ksum.tensor, offset=ksum.offset + qt * (KC * 128),
                    ap=[ksum.ap[0], [1, 128], [128, KC]])
                qsl = qsum[:, qt]
                nc.vector.tensor_reduce(
                    bass.AP(tensor=qsl.tensor, offset=qsl.offset,
                            ap=[*qsl.ap, [1, 1]]), kv,
                    axis=mybir.AxisListType.X, op=ADD)
                nc.sync.dma_start(
                    out=qsumT[:, qt : qt + 1], in_=qsum[0:1, qt, :])
                # context
                for kb in range(KC):
                    nc.tensor.matmul(
                        pctx[:, 0:H], lhsT=wtsT[:, qt, kb, :],
                        rhs=hidb[:, kb, :], start=(kb == 0), stop=(kb == KC - 1))
                nc.vector.reciprocal(rinv[:, qt : qt + 1], qsumT[:, qt : qt + 1])
                nc.vector.tensor_scalar_mul(
                    octx[:, qt, :], pctx[:, 0:H], rinv[:, qt : qt + 1])
                nc.sync.dma_start(
                    out=out[qt * 128 : (qt + 1) * 128, :], in_=octx[:, qt, :])
                if dbg is not None and rep == 0:
                    dsc = sg.tile([128, S], f32, tag=f"dsc{qt}")
                    nc.vector.tensor_copy(
                        dsc, scT[:, qt].rearrange("p a b -> p (a b)"))
                    nc.sync.dma_start(
                        out=dbg[qt * 128 : (qt + 1) * 128, :], in_=dsc)

    nc.compile()
    return nc


def _get(reps=1, skip=()):
    key = (reps, tuple(skip))
    if key not in _CACHE:
        _CACHE[key] = _build(reps, skip)
    return _CACHE[key]


def _to_bf16(a):
    import ml_dtypes
    return np.ascontiguousarray(a).astype(ml_dtypes.bfloat16)


def _in_maps(hs, W, U, v):
    hs = np.asarray(hs, np.float32)
    WTh = np.asarray(W, np.float32).T
    UTh = np.asarray(U, np.float32).T
    WTp = np.ascontiguousarray(WTh.reshape(2, HP, H).transpose(1, 0, 2))
    UTp = np.ascontiguousarray(UTh.reshape(2, HP, H).transpose(1, 0, 2))
    vrep = np.tile(np.asarray(v, np.float32)[None, :], (HP, 1))
    maps = []
    for c in range(NCORES):
        b, qh = divmod(c, 2)
        hT = hs[b].T
        hidT = np.ascontiguousarray(hT.reshape(2, HP, S).transpose(1, 0, 2))
        hqT = np.ascontiguousarray(
            hT[:, qh * QPC : (qh + 1) * QPC].reshape(2, HP, QPC).transpose(1, 0, 2))
        hidb = hs[b].reshape(KC, HP, H).transpose(1, 0, 2)
        maps.append({
            "hidT_u": hidT,
            "hqT_u": hqT,
            "hidb_u": hidb.astype(np.float32),
            "WTp_u": WTp,
            "UTp_u": UTp,
            "vrep_u": _to_bf16(vrep),
        })
    return maps


def run(hidden_states, W, U, v, reps=1, skip=()):
    from concourse.bass_utils import run_bass_kernel_spmd

    nc = _get(reps, skip)
    res = run_bass_kernel_spmd(
        nc, _in_maps(hidden_states, W, U, v), core_ids=list(range(NCORES)))
    ctxout = np.empty((B, S, H), np.float32)
    for c in range(NCORES):
        b, qh = divmod(c, 2)
        ctxout[b, qh * QPC : (qh + 1) * QPC] = res.results[c]["out"]
    return ctxout


def kernel(**inputs):
    return run(inputs["hidden_states"], inputs["W"], inputs["U"], inputs["v"])
